# revision 1
# baseline (speedup 1.0000x reference)
"""Trainium2 Bass kernel for nn_BivectorPhasorBlock.

Strategy:
- 8 cores = data-parallel over B (2) x sequence-parallel over L (4 chunks of
  1024 tokens). Cross-shard cumsum carry via a 4KB AllGather + per-core mask.
- On-device layout: features on partitions, tokens on the free dim, so every
  matmul uses the natural [in,out] weight as lhsT and the L-cumsum is a
  tensor_tensor_scan along the free dim (carry = per-partition scan initial).
- Host preprocessing: permute wk2/wq2/wv columns (and ln_g/ln_b/wo rows) to
  component-major order; transpose x per shard; cast weights to bf16.
- Math folds: the *pi/2 angle scale is folded into the Sin activation scale;
  the 1/sqrt(t+1) normalization is dropped (LayerNorm is scale-invariant per
  token up to a negligible eps term).
"""

import sys
from contextlib import ExitStack

for _p in ("/opt/trn_rl_repo", "/root/.axon_site/_ro/trn_rl_repo"):
    if _p not in sys.path:
        sys.path.append(_p)

import numpy as np
import ml_dtypes

import concourse.bass as bass
import concourse.tile as tile
from concourse import bacc, mybir
from concourse.bass_utils import run_bass_kernel_spmd

fp32 = mybir.dt.float32
bf16 = mybir.dt.bfloat16
AF = mybir.ActivationFunctionType
ALU = mybir.AluOpType

B, L, D = 2, 4096, 1024
K = D // 4          # 256
AD = 6 * K          # 1536 angle features
NCORES = 8
NB_L = NCORES // B  # L-chunks per batch = 4

DP = D // 128       # 8 feature ptiles
APT = AD // 128     # 12 angle ptiles
HALF_PI = 1.5707963267948966
QUARTER_PI = 0.7853981633974483
GELU_AF = None  # resolved at build time; simtest overrides to Tanh


def _build(Lc, T):
    """Build + compile the SPMD program for one core (Lc tokens, chunk T)."""
    NCH = Lc // T
    assert Lc % T == 0

    nc = bacc.Bacc("TRN2", target_bir_lowering=False, debug=False,
                   num_devices=NCORES)

    dr = {}
    def din(name, shape, dt):
        dr[name] = nc.dram_tensor(name, shape, dt, kind="ExternalInput")
    din("xbf", [D, Lc], bf16)
    din("wk1", [D, D], bf16)
    din("wk2", [D, AD], bf16)
    din("wq1", [D, D], bf16)
    din("wq2", [D, AD], bf16)
    din("wv", [D, D], bf16)
    din("wo", [D, D], bf16)
    din("bk1", [D, 1], fp32)
    din("bk2", [AD, 1], fp32)
    din("bq1", [D, 1], fp32)
    din("bq2", [AD, 1], fp32)
    din("bvr", [1, D], bf16)
    din("bo", [D, 1], fp32)
    din("lng", [D, 1], fp32)
    din("lnb", [D, 1], fp32)
    din("mask", [128, NCORES, DP], fp32)
    dr["out"] = nc.dram_tensor("out", [D, Lc], bf16, kind="ExternalOutput")

    with tile.TileContext(nc) as tc:
        _body(nc, tc, dr, Lc, T, NCH)
    nc.compile()
    return nc


def _rotor_apply(nc, rt, cb, a, v, out_tiles, accs, T, reverse):
    """Emit rotor construction + application for one khalf / chunk.

    a: 6 [128,T] bf16 angle tiles (raw tanh outputs, pi/2 folded out)
    v: 4 [128,T] bf16 input tiles
    out_tiles: 4 [128,T] bf16 AP destinations
    accs: None or 4 [128,1] fp32 accum_out APs (free-dim sums)
    reverse: False for key rotor (+biv), True for query rotor (-biv)
    rt: callable returning a fresh [128,T] bf16 ring tile
    """
    # squares on DVE (keeps the block start engine-local)
    sq = []
    for i in range(6):
        s_ = rt()
        nc.vector.tensor_mul(s_, a[i], a[i])
        sq.append(s_)
    # mag2 = sum of squares (DVE tree)
    t01 = rt(); nc.vector.tensor_add(t01, sq[0], sq[1])
    t23 = rt(); nc.vector.tensor_add(t23, sq[2], sq[3])
    t45 = rt(); nc.vector.tensor_add(t45, sq[4], sq[5])
    t03 = rt(); nc.vector.tensor_add(t03, t01, t23)
    mag2 = rt(); nc.vector.tensor_add(mag2, t03, t45)
    # ACT transcendental chain kicked off early; the ACT-independent W and
    # c_i products below hide its latency.
    # magr = sqrt(mag2 + 1e-16)  (clamp folded into bias)
    magr = rt()
    nc.scalar.activation(magr, mag2, AF.Sqrt, bias=cb['eps16'][:, 0:1])
    # half-angle scheme keeps Sin args in the ScalarE-valid [-pi, pi]:
    # sh = sin(pi/4 m), ch = cos(pi/4 m) = sin(pi/2 - pi/4 m)
    # sinm = sin(pi/2 m) = 2 sh ch ; s = cos(pi/2 m) = 1 - 2 sh^2
    sh = rt()
    nc.scalar.activation(sh, magr, AF.Sin, scale=QUARTER_PI)
    ch = rt()
    nc.scalar.activation(ch, magr, AF.Sin, bias=cb['hpi'][:, 0:1],
                         scale=-QUARTER_PI)
    # W_i: the antisymmetric matvec up to sign (sigma = [+,+,+,-])
    # W1 = (a0 v2 + a1 v3) + a2 v4
    # W2 = (a3 v3 + a4 v4) - a0 v1
    # W3 = (a5 v4 - a1 v1) - a3 v2
    # W4 = (a2 v1 + a4 v2) + a5 v3   [true w4 = -W4]
    specs = [
        ((0, 1), (1, 2), ALU.add, (2, 3), ALU.add),
        ((3, 2), (4, 3), ALU.add, (0, 0), ALU.subtract),
        ((5, 3), (1, 0), ALU.subtract, (3, 1), ALU.subtract),
        ((2, 0), (4, 1), ALU.add, (5, 2), ALU.add),
    ]
    sigs = [1, 1, 1, -1]
    Ws = []
    for (p1, p2, opa, p3, opb) in specs:
        ma = rt(); nc.vector.tensor_mul(ma, a[p1[0]], v[p1[1]])
        mb = rt(); nc.vector.tensor_mul(mb, a[p2[0]], v[p2[1]])
        s1 = rt(); nc.vector.tensor_tensor(s1, ma, mb, opa)
        mc = rt(); nc.vector.tensor_mul(mc, a[p3[0]], v[p3[1]])
        w_ = rt(); nc.vector.tensor_tensor(w_, s1, mc, opb)
        Ws.append(w_)
    # c_i sums of squares (bivector order: t12,t13,t14,t23,t24,t34)
    c_idx = [(0, 1, 2), (0, 3, 4), (1, 3, 5), (2, 4, 5)]
    cs = []
    for (x_, y_, z_) in c_idx:
        e = rt(); nc.vector.tensor_add(e, sq[x_], sq[y_])
        c = rt(); nc.vector.tensor_add(c, e, sq[z_])
        cs.append(c)
    # resume the ACT-dependent tail (sh/ch ready by now)
    sh2 = rt(); nc.vector.tensor_mul(sh2, sh, sh)
    s = rt()
    nc.vector.tensor_scalar(s, sh2, -2.0, 1.0, ALU.mult, ALU.add)
    sinm = rt()
    nc.vector.scalar_tensor_tensor(sinm, sh, 2.0, ch, ALU.mult, ALU.mult)
    invm = rt()
    with nc.allow_low_precision("bf16 reciprocal fine for 2e-2 tol"):
        nc.vector.reciprocal(invm, magr)
    sinc = rt(); nc.vector.tensor_mul(sinc, sinm, invm)
    p2s = rt()
    nc.vector.scalar_tensor_tensor(p2s, s, 2.0, sinc, ALU.mult, ALU.mult)
    q = rt(); nc.vector.tensor_mul(q, sinc, sinc)
    s2 = rt(); nc.vector.tensor_mul(s2, s, s)
    # out_i = (s2 - q*c_i) * v_i +/- p2s * W_i
    for i in range(4):
        qc = rt(); nc.vector.tensor_mul(qc, q, cs[i])
        d = rt(); nc.vector.tensor_sub(d, s2, qc)
        dv = rt(); nc.vector.tensor_mul(dv, d, v[i])
        pw = rt(); nc.vector.tensor_mul(pw, p2s, Ws[i])
        sign = sigs[i] * (-1 if reverse else 1)
        op = ALU.add if sign > 0 else ALU.subtract
        if accs is not None:
            nc.vector.scalar_tensor_tensor(out_tiles[i], dv, 0.0, pw,
                                           ALU.add, op, accum_out=accs[i])
        else:
            nc.vector.scalar_tensor_tensor(out_tiles[i], dv, 0.0, pw,
                                           ALU.add, op)


def _body(nc, tc, dr, Lc, T, NCH):
    gelu_af = GELU_AF if GELU_AF is not None else AF.Gelu
    ctx = ExitStack()
    consts = ctx.enter_context(tc.tile_pool(name="consts", bufs=1))
    misc = ctx.enter_context(tc.tile_pool(name="misc", bufs=1))
    tmp = ctx.enter_context(tc.tile_pool(name="tmp", bufs=22))
    rotp = ctx.enter_context(tc.tile_pool(name="rot", bufs=1))
    pmm = ctx.enter_context(tc.tile_pool(name="pmm", bufs=4, space="PSUM"))
    pln = ctx.enter_context(tc.tile_pool(name="pln", bufs=1, space="PSUM"))
    pbc = ctx.enter_context(tc.tile_pool(name="pbc", bufs=1, space="PSUM"))
    dram = ctx.enter_context(tc.tile_pool(name="dram", bufs=1, space="DRAM"))

    def rt():
        return tmp.tile([128, T], bf16, tag="rt", name="rt")

    # ---- constants / biases ----
    ones_col = consts.tile([128, 1], bf16)
    nc.gpsimd.memset(ones_col, 1.0)
    ones_row = consts.tile([1, 128], bf16)
    nc.gpsimd.memset(ones_row, 1.0)
    ones_T = consts.tile([1, T], bf16)
    nc.gpsimd.memset(ones_T, 1.0)
    zeros_T = consts.tile([128, T], bf16)
    nc.gpsimd.memset(zeros_T, 0.0)
    c_eps16 = consts.tile([128, 1], fp32)
    nc.gpsimd.memset(c_eps16, 1e-16)
    c_hpi = consts.tile([128, 1], fp32)
    nc.gpsimd.memset(c_hpi, HALF_PI)
    c_eps5 = consts.tile([1, 1], fp32)
    nc.gpsimd.memset(c_eps5, 1e-5)
    cb = {'eps16': c_eps16, 'hpi': c_hpi}

    def load_bias(name, n):
        t_ = consts.tile([128, n], fp32, tag=f"b_{name}")
        nc.sync.dma_start(t_, dr[name][:, :].rearrange("(m p) o -> p (m o)",
                                                       p=128))
        return t_

    bk1_sb = load_bias("bk1", DP)
    bk2_sb = load_bias("bk2", APT)
    bq1_sb = load_bias("bq1", DP)
    bq2_sb = load_bias("bq2", APT)
    bo_sb = load_bias("bo", DP)
    lng_sb = load_bias("lng", DP)
    lnb_sb = load_bias("lnb", DP)
    bvr_sb = consts.tile([1, D], bf16)
    nc.sync.dma_start(bvr_sb, dr["bvr"][:, :])
    mask_sb = misc.tile([128, NCORES, DP], fp32)
    nc.sync.dma_start(mask_sb, dr["mask"][:, :, :])

    def mm_layer(m_tiles, k_tiles, w_sb, rhs_tiles, sl, bias_row=None):
        """psum[m] = sum_k w_sb[k][:, m-tile].T @ rhs_tiles[k][:, sl]."""
        outs = []
        for m in range(m_tiles):
            ps = pmm.tile([128, T], fp32, tag="mm")
            for k in range(k_tiles):
                last = (k == k_tiles - 1) and bias_row is None
                nc.tensor.matmul(ps, w_sb[k][:, m * 128:(m + 1) * 128],
                                 rhs_tiles[k][:, sl],
                                 start=(k == 0), stop=last)
            if bias_row is not None:
                nc.tensor.matmul(ps, bias_row[:, m * 128:(m + 1) * 128],
                                 ones_T, start=False, stop=True)
            outs.append(ps)
        return outs

    rot = [rotp.tile([128, Lc], bf16, tag=f"rot{f}", name=f"rot{f}") for f in range(DP)]
    accs = [[misc.tile([128, 1], fp32, tag=f"acc{f}_{ch}", name=f"acc{f}_{ch}")
             for f in range(DP)] for ch in range(NCH)]

    # ======== x load (resident for phases 1, 3a, and the residual) ========
    xp = ctx.enter_context(tc.tile_pool(name="xp", bufs=1))
    if True:
        xbf_sb = []
        for p in range(DP):
            t_ = xp.tile([128, Lc], bf16, tag=f"xbf{p}")
            eng = nc.sync if p % 2 == 0 else nc.gpsimd
            eng.dma_start(t_, dr["xbf"][p * 128:(p + 1) * 128, :])
            xbf_sb.append(t_)

        # -------- phase 1: key angles, value, rotated --------
        es_wq = ExitStack()
        es_wk = ExitStack()
        wqp = es_wq.enter_context(tc.tile_pool(name="wq", bufs=1))
        wkp = es_wk.enter_context(tc.tile_pool(name="wk", bufs=1))
        p1w = es_wk.enter_context(tc.tile_pool(name="p1w", bufs=1))
        if True:
            wk1_sb = [wkp.tile([128, D], bf16, tag=f"wk1_{k}", name=f"wk1_{k}")
                      for k in range(DP)]
            wk2_sb = [wkp.tile([128, AD], bf16, tag=f"wk2_{k}", name=f"wk2_{k}")
                      for k in range(DP)]
            wv_sb = [wkp.tile([128, D], bf16, tag=f"wv_{k}", name=f"wv_{k}")
                     for k in range(DP)]
            for k in range(DP):
                nc.sync.dma_start(wk1_sb[k], dr["wk1"][k * 128:(k + 1) * 128, :])
                nc.gpsimd.dma_start(wk2_sb[k],
                                    dr["wk2"][k * 128:(k + 1) * 128, :])
                nc.scalar.dma_start(wv_sb[k], dr["wv"][k * 128:(k + 1) * 128, :])
            # prefetch query-path weights during phase-1 compute (fresh space)
            wq1_sb = [wqp.tile([128, D], bf16, tag=f"wq1_{k}", name=f"wq1_{k}")
                      for k in range(DP)]
            wq2_sb = [wqp.tile([128, AD], bf16, tag=f"wq2_{k}", name=f"wq2_{k}")
                      for k in range(DP)]
            for k in range(DP):
                nc.sync.dma_start(wq1_sb[k], dr["wq1"][k * 128:(k + 1) * 128, :])
                nc.gpsimd.dma_start(wq2_sb[k],
                                    dr["wq2"][k * 128:(k + 1) * 128, :])

            for ch in range(NCH):
                sl = slice(ch * T, (ch + 1) * T)
                ps = mm_layer(DP, DP, wk1_sb, xbf_sb, sl)
                hk = []
                for m in range(DP):
                    h_ = p1w.tile([128, T], bf16, tag=f"hk{m}")
                    nc.scalar.activation(h_, ps[m], gelu_af,
                                         bias=bk1_sb[:, m:m + 1])
                    hk.append(h_)
                ps = mm_layer(APT, DP, wk2_sb, hk, slice(0, T))
                ak = []
                for m in range(APT):
                    a_ = p1w.tile([128, T], bf16, tag=f"ak{m}")
                    nc.scalar.activation(a_, ps[m], AF.Tanh,
                                         bias=bk2_sb[:, m:m + 1])
                    ak.append(a_)
                ps = mm_layer(DP, DP, wv_sb, xbf_sb, sl, bias_row=bvr_sb)
                vt = []
                for m in range(DP):
                    v_ = p1w.tile([128, T], bf16, tag=f"v{m}")
                    nc.scalar.activation(v_, ps[m], AF.Copy)
                    vt.append(v_)
                for h in range(2):
                    a6 = [ak[2 * i + h] for i in range(6)]
                    v4 = [vt[2 * i + h] for i in range(4)]
                    outs = [rot[2 * i + h][:, sl] for i in range(4)]
                    acc4 = [accs[ch][2 * i + h] for i in range(4)]
                    _rotor_apply(nc, rt, cb, a6, v4, outs, acc4, T,
                                 reverse=False)

        es_wk.close()

        # -------- phase 2: cross-core carry --------
        sums = misc.tile([128, DP], fp32)
        for f in range(DP):
            acc_total = accs[0][f]
            for ch in range(1, NCH):
                nt = misc.tile([128, 1], fp32, tag=f"acct{f}_{ch}")
                nc.vector.tensor_add(nt, acc_total, accs[ch][f])
                acc_total = nt
            nc.vector.tensor_copy(sums[:, f:f + 1], acc_total)
        cc_in = dram.tile([128, DP], fp32)
        cc_out = dram.tile([NCORES * 128, DP], fp32)
        nc.sync.dma_start(cc_in, sums)
        nc.gpsimd.collective_compute(
            "AllGather", ALU.bypass, replica_groups=[list(range(NCORES))],
            ins=[cc_in.opt()], outs=[cc_out.opt()])
        g = misc.tile([128, NCORES, DP], fp32)
        nc.sync.dma_start(g, cc_out[:, :].rearrange("(c p) f -> p c f", p=128))
        gm = misc.tile([128, NCORES, DP], fp32)
        nc.vector.tensor_mul(gm, g, mask_sb)
        t1 = misc.tile([128, 4, DP], fp32)
        nc.vector.tensor_add(t1, gm[:, 0:4, :], gm[:, 4:8, :])
        t2 = misc.tile([128, 2, DP], fp32)
        nc.vector.tensor_add(t2, t1[:, 0:2, :], t1[:, 2:4, :])
        carry = misc.tile([128, 1, DP], fp32)
        nc.vector.tensor_add(carry, t2[:, 0:1, :], t2[:, 1:2, :])

        # -------- phase 3a: query angles (weights prefetched) --------
        es_wo = ExitStack()
        wop_pool = es_wo.enter_context(tc.tile_pool(name="wop", bufs=1))
        wo_sb = [wop_pool.tile([128, D], bf16, tag=f"wo_{k}", name=f"wo_{k}")
                 for k in range(DP)]
        for k in range(DP):
            eng = nc.sync if k % 2 == 0 else nc.gpsimd
            eng.dma_start(wo_sb[k], dr["wo"][k * 128:(k + 1) * 128, :])
        es_aq = ExitStack()
        aqp = es_aq.enter_context(tc.tile_pool(name="aqp", bufs=2))
        aq_all = [[aqp.tile([128, T], bf16, tag=f"aq{m}", name=f"aq{m}")
                   for m in range(APT)] for ch in range(NCH)]
        with tc.tile_pool(name="p3aw", bufs=1) as p3aw:
            for ch in range(NCH):
                sl = slice(ch * T, (ch + 1) * T)
                ps = mm_layer(DP, DP, wq1_sb, xbf_sb, sl)
                hq = []
                for m in range(DP):
                    h_ = p3aw.tile([128, T], bf16, tag=f"hq{m}")
                    nc.scalar.activation(h_, ps[m], gelu_af,
                                         bias=bq1_sb[:, m:m + 1])
                    hq.append(h_)
                ps = mm_layer(APT, DP, wq2_sb, hq, slice(0, T))
                for m in range(APT):
                    nc.scalar.activation(aq_all[ch][m], ps[m], AF.Tanh,
                                         bias=bq2_sb[:, m:m + 1])

    # ======== phase 3b: scan, retrieve, LN, output ========
    with tc.tile_pool(name="p3bw", bufs=1) as p3bw, \
         tc.tile_pool(name="p3bs", bufs=1) as p3bs:

        carries = [carry]
        for ch in range(1, NCH):
            cn = misc.tile([128, 1, DP], fp32, tag=f"carry{ch}",
                           name=f"carry{ch}")
            for f in range(DP):
                nc.vector.tensor_add(cn[:, 0, f:f + 1],
                                     carries[-1][:, 0, f:f + 1],
                                     accs[ch - 1][f])
            carries.append(cn)
        for ch in range(NCH):
            sl = slice(ch * T, (ch + 1) * T)
            mem = []
            for f in range(DP):
                m_ = p3bw.tile([128, T], bf16, tag=f"mem{f}", bufs=1)
                nc.vector.tensor_tensor_scan(m_, rot[f][:, sl], zeros_T,
                                             carries[ch][:, 0, f:f + 1],
                                             ALU.add, ALU.add)
                mem.append(m_)
            retr = [p3bw.tile([128, T], bf16, tag=f"retr{f}", name=f"retr{f}")
                    for f in range(DP)]
            for h in range(2):
                a6 = [aq_all[ch][2 * i + h] for i in range(6)]
                m4 = [mem[2 * i + h] for i in range(4)]
                outs = [retr[2 * i + h][:, :] for i in range(4)]
                _rotor_apply(nc, rt, cb, a6, m4, outs, None, T, reverse=True)
            # LN stats via PE reduction over feature partitions
            ps_sum = pln.tile([1, T], fp32, tag="lnsum")
            ps_ss = pln.tile([1, T], fp32, tag="lnss")
            rsqs = []
            for f in range(DP):
                rs_ = rt()
                nc.scalar.activation(rs_, retr[f], AF.Square)
                rsqs.append(rs_)
            for f in range(DP):
                nc.tensor.matmul(ps_sum, ones_col, retr[f],
                                 start=(f == 0), stop=(f == DP - 1))
            for f in range(DP):
                nc.tensor.matmul(ps_ss, ones_col, rsqs[f],
                                 start=(f == 0), stop=(f == DP - 1))
            mu = p3bs.tile([1, T], fp32, tag="mu")
            nc.vector.tensor_scalar_mul(mu, ps_sum, 1.0 / D)
            musq = p3bs.tile([1, T], fp32, tag="musq")
            nc.vector.tensor_mul(musq, mu, mu)
            dv_ = p3bs.tile([1, T], fp32, tag="var")
            nc.vector.scalar_tensor_tensor(dv_, musq, -float(D), ps_ss,
                                           ALU.mult, ALU.add)
            std = p3bs.tile([1, T], fp32, tag="std")
            nc.scalar.activation(std, dv_, AF.Sqrt, bias=c_eps5[:, 0:1],
                                 scale=1.0 / D)
            istd = p3bs.tile([1, T], fp32, tag="istd")
            nc.vector.reciprocal(istd, std)
            bt = p3bs.tile([1, T], fp32, tag="bt")
            nc.vector.tensor_mul(bt, mu, istd)
            istd_bf = p3bs.tile([1, T], bf16, tag="istdbf")
            nc.scalar.activation(istd_bf, istd, AF.Copy)
            bt_bf = p3bs.tile([1, T], bf16, tag="btbf")
            nc.scalar.activation(bt_bf, bt, AF.Copy)
            ps_a = pbc.tile([128, T], fp32, tag="bcA")
            nc.tensor.matmul(ps_a, ones_row, istd_bf, start=True, stop=True)
            ps_b = pbc.tile([128, T], fp32, tag="bcB")
            nc.tensor.matmul(ps_b, ones_row, bt_bf, start=True, stop=True)
            a_b = p3bs.tile([128, T], bf16, tag="Ab")
            nc.scalar.activation(a_b, ps_a, AF.Copy)
            b_b = p3bs.tile([128, T], bf16, tag="Bb")
            nc.scalar.activation(b_b, ps_b, AF.Copy)
            # rn = (retr*istd - mu*istd)*g + b
            rn = []
            for f in range(DP):
                z1 = rt()
                nc.vector.tensor_mul(z1, retr[f], a_b)
                z2 = rt()
                nc.vector.tensor_sub(z2, z1, b_b)
                rn_ = p3bw.tile([128, T], bf16, tag=f"rn{f}")
                nc.vector.tensor_scalar(rn_, z2, lng_sb[:, f:f + 1],
                                        lnb_sb[:, f:f + 1], ALU.mult, ALU.add)
                rn.append(rn_)
            # out = x + rn @ wo + bo
            ps = mm_layer(DP, DP, wo_sb, rn, slice(0, T))
            for m in range(DP):
                o_ = p3bw.tile([128, T], bf16, tag=f"o{m}")
                nc.vector.scalar_tensor_tensor(o_, ps[m], bo_sb[:, m:m + 1],
                                               xbf_sb[m][:, sl], ALU.add,
                                               ALU.add)
                eng = nc.sync if m % 2 == 0 else nc.gpsimd
                eng.dma_start(dr["out"][m * 128:(m + 1) * 128, sl], o_)

    es_aq.close()
    es_wo.close()
    es_wq.close()
    ctx.close()


# ============================ host side ============================

_PERM6 = np.array([k * 6 + i for i in range(6) for k in range(K)])
_PERM4 = np.array([k * 4 + i for i in range(4) for k in range(K)])


def _prep_weights(wk1, bk1, wk2, bk2, wq1, bq1, wq2, bq2, wv, bv,
                  ln_g, ln_b, wo, bo):
    b16 = ml_dtypes.bfloat16
    col = lambda a: np.ascontiguousarray(
        np.asarray(a, np.float32)).reshape(-1, 1)
    d = {
        "wk1": np.asarray(wk1, np.float32).astype(b16),
        "wk2": np.asarray(wk2, np.float32)[:, _PERM6].astype(b16),
        "wq1": np.asarray(wq1, np.float32).astype(b16),
        "wq2": np.asarray(wq2, np.float32)[:, _PERM6].astype(b16),
        "wv": np.asarray(wv, np.float32)[:, _PERM4].astype(b16),
        "wo": np.ascontiguousarray(
            np.asarray(wo, np.float32)[_PERM4, :]).astype(b16),
        "bk1": col(bk1), "bq1": col(bq1),
        "bk2": col(np.asarray(bk2, np.float32)[_PERM6]),
        "bq2": col(np.asarray(bq2, np.float32)[_PERM6]),
        "bvr": np.asarray(bv, np.float32)[_PERM4].reshape(1, -1).astype(b16),
        "bo": col(bo),
        "lng": col(np.asarray(ln_g, np.float32)[_PERM4]),
        "lnb": col(np.asarray(ln_b, np.float32)[_PERM4]),
    }
    return {k: np.ascontiguousarray(v) for k, v in d.items()}


def _make_in_maps(x, wd, Lc):
    b16 = ml_dtypes.bfloat16
    x = np.asarray(x, np.float32)
    in_maps = []
    for c in range(NCORES):
        b, j = c // NB_L, c % NB_L
        xs = np.ascontiguousarray(x[b, j * Lc:(j + 1) * Lc, :].T)  # [D, Lc]
        m8 = np.zeros((NCORES,), np.float32)
        for c2 in range(NCORES):
            if c2 // NB_L == b and c2 % NB_L < j:
                m8[c2] = 1.0
        maskrep = np.ascontiguousarray(
            np.broadcast_to(m8[None, :, None], (128, NCORES, DP))
        ).astype(np.float32)
        im = dict(wd)
        im["xbf"] = xs.astype(b16)
        im["mask"] = maskrep
        in_maps.append(im)
    return in_maps


_CACHE = {}


def _get_nc(Lc, T):
    key = (Lc, T)
    if key not in _CACHE:
        _CACHE[key] = _build(Lc, T)
    return _CACHE[key]


def _enable_compile_cache():
    try:
        import jax, tempfile, os
        cdir = os.path.join(tempfile.gettempdir(), "bass_jax_cache")
        os.makedirs(cdir, exist_ok=True)
        jax.config.update("jax_compilation_cache_dir", cdir)
        jax.config.update("jax_persistent_cache_min_compile_time_secs", 0.0)
        jax.config.update("jax_persistent_cache_min_entry_size_bytes", 0)
    except Exception:
        pass


def run(x, weights, Lc, T, trace=False):
    _enable_compile_cache()
    nc = _get_nc(Lc, T)
    wd = _prep_weights(**weights)
    in_maps = _make_in_maps(x, wd, Lc)
    res = run_bass_kernel_spmd(nc, in_maps, core_ids=list(range(NCORES)),
                               trace=trace)
    x = np.asarray(x, np.float32)
    out = np.empty_like(x)
    for c in range(NCORES):
        b, j = c // NB_L, c % NB_L
        out[b, j * Lc:(j + 1) * Lc, :] = np.asarray(
            res.results[c]["out"], np.float32).T
    return out, res


def kernel(x, wk1, bk1, wk2, bk2, wq1, bq1, wq2, bq2, wv, bv,
           ln_g, ln_b, wo, bo):
    weights = dict(wk1=wk1, bk1=bk1, wk2=wk2, bk2=bk2, wq1=wq1, bq1=bq1,
                   wq2=wq2, bq2=bq2, wv=wv, bv=bv, ln_g=ln_g, ln_b=ln_b,
                   wo=wo, bo=bo)
    out, _ = run(x, weights, Lc=L // NB_L, T=512)
    return out.astype(np.float32)

